# revision 1
# baseline (speedup 1.0000x reference)
"""GPT-style dense transformer on 8 Trainium2 NeuronCores.

Sharding: token-parallel. Core c owns positions t = 8*i + c of BOTH batches
(256 positions per batch at full size -> 512 tokens per core). All per-token
work (LN, qkv, out_proj, ff, lm_head) is local; attention needs all keys, so
K^T and V are AllGathered across the 8 cores once per layer (bf16, ~1.5MB per
rank). The strided assignment makes every core's causal structure identical
(block-lower-triangular over local indices, with a per-source-core diagonal
rule c' <= c shipped as a data mask), so one SPMD program serves all cores.

Layout trick: the residual stream lives TRANSPOSED in SBUF as x^T [D, tokens].
Every matmul then consumes natural-layout weights as the stationary operand
and transposed activations as the moving operand, producing transposed
activations again -- zero on-device transposes after the embedding load.
Attention is computed as S^T = K^T.T @ Q^T (scores with keys on partitions),
exp'd without max-subtraction (scores are bounded ~|0.3| by construction),
masked multiplicatively, and y^T = (V|1).T @ expS^T accumulates both the
numerator and the softmax denominator (ones column) in one PSUM pass.
LN scale/bias are folded into the adjacent weights on the host.
"""

import sys

for _p in ("/opt/trn_rl_repo",):
    if _p not in sys.path:
        sys.path.insert(0, _p)

import numpy as np
import ml_dtypes

import concourse.bass as bass
import concourse.bacc as bacc
import concourse.mybir as mybir
import concourse.tile as tile
from concourse.masks import make_identity

BF16 = mybir.dt.bfloat16
F32 = mybir.dt.float32
I32 = mybir.dt.int32
AF = mybir.ActivationFunctionType
ALU = mybir.AluOpType

NCORES = 8
H = 12          # heads
HD = 64         # head dim
D = 768
D3 = 3 * D      # 2304
DF = 4 * D      # 3072
KD = D // 128   # 6 d-tiles
EPS = 1e-5

bf16 = ml_dtypes.bfloat16


def build_nc(nb, L, V, stop_at=None):
    """Build the SPMD Bass module. nb = 128-token tiles per (core, batch).
    Full size: nb=2 -> 512 tokens/core, T = 8*128*nb = 2048."""
    NT = 2 * nb          # token tiles per core
    PT = NT * 128        # tokens per core
    NVC = (V + 511) // 512  # vocab chunks for lm_head

    nc = bacc.Bacc("TRN2", target_bir_lowering=False, num_devices=NCORES)

    # ---- I/O ----
    idxs = nc.dram_tensor("idxs", [128, NT], I32, kind="ExternalInput")
    posT = nc.dram_tensor("posT", [D, PT], F32, kind="ExternalInput")
    masks = nc.dram_tensor("masks", [128, NCORES * 128], BF16, kind="ExternalInput")
    toke = nc.dram_tensor("toke", [V, D], F32, kind="ExternalInput")
    embT = nc.dram_tensor("embT", [D, V], BF16, kind="ExternalInput")
    wqkv = [nc.dram_tensor(f"wqkv{l}", [D, D3], BF16, kind="ExternalInput") for l in range(L)]
    bqkv = [nc.dram_tensor(f"bqkv{l}", [128, 12], F32, kind="ExternalInput") for l in range(L)]
    bqv = [nc.dram_tensor(f"bqv{l}", [1, D], F32, kind="ExternalInput") for l in range(L)]
    wout = [nc.dram_tensor(f"wout{l}", [D, D], BF16, kind="ExternalInput") for l in range(L)]
    w1 = [nc.dram_tensor(f"w1_{l}", [D, DF], BF16, kind="ExternalInput") for l in range(L)]
    b1 = [nc.dram_tensor(f"b1_{l}", [128, 24], F32, kind="ExternalInput") for l in range(L)]
    w2 = [nc.dram_tensor(f"w2_{l}", [DF, D], BF16, kind="ExternalInput") for l in range(L)]
    logits = nc.dram_tensor("logits", [PT, V], F32, kind="ExternalOutput")

    from contextlib import ExitStack
    with tile.TileContext(nc) as tc, ExitStack() as ctx:
        def pool(**kw):
            return ctx.enter_context(tc.tile_pool(**kw))
        # ---- pools ----
        const = pool(name="const", bufs=1)
        resid = pool(name="resid", bufs=1)
        acts = pool(name="acts", bufs=1)
        kvres = pool(name="kvres", bufs=1)
        wpool = pool(name="wpool", bufs=1)
        biasp = pool(name="biasp", bufs=2)
        rot = pool(name="rot", bufs=2)
        esp = pool(name="esp", bufs=6)
        gp = pool(name="gp", bufs=4)
        embp = pool(name="embp", bufs=2)
        logp = pool(name="logp", bufs=3)
        rowp = pool(name="rowp", bufs=4)
        ps_s = pool(name="ps_s", bufs=2, space="PSUM")
        ps_y = pool(name="ps_y", bufs=4, space="PSUM")
        ps_m = pool(name="ps_m", bufs=2, space="PSUM")
        dram = pool(name="dram", bufs=2, space="DRAM")

        # ---- constants ----
        ident = const.tile([128, 128], F32, name="ident", tag="ident")
        make_identity(nc, ident)
        ones_col = const.tile([128, 1], BF16, name="ones_col", tag="ones_col")
        nc.gpsimd.memset(ones_col[:, :], 1.0)
        ones_row = const.tile([1, 128], F32, name="ones_row", tag="ones_row")
        nc.gpsimd.memset(ones_row[:, :], 1.0)
        eps_t = const.tile([1, 1], F32, name="eps_t", tag="eps_t")
        nc.gpsimd.memset(eps_t[:, :], EPS)
        zero_col = const.tile([128, 1], F32, name="zero_col", tag="zero_col")
        nc.gpsimd.memset(zero_col[:, :], 0.0)
        mask_sb = const.tile([128, NCORES * 128], BF16, name="mask_sb", tag="mask_sb")
        nc.sync.dma_start(out=mask_sb[:, :], in_=masks[:, :])
        idx_sb = const.tile([128, NT], I32, name="idx_sb", tag="idx_sb")
        nc.sync.dma_start(out=idx_sb[:, :], in_=idxs[:, :])

        # ---- persistent per-layer state ----
        xT = [resid.tile([128, PT], F32, name=f"xt{d}", tag=f"xt{d}") for d in range(KD)]
        hT = [acts.tile([128, PT], BF16, name=f"ht{d}", tag=f"ht{d}") for d in range(KD)]
        qT = [acts.tile([128, PT], BF16, name=f"qt{d}", tag=f"qt{d}") for d in range(KD)]
        yT = [acts.tile([128, PT], BF16, name=f"yt{d}", tag=f"yt{d}") for d in range(KD)]
        # gathered K^T (half the heads at a time): [c'][r] covering 384 rows
        ktg = [[kvres.tile([128, PT], BF16, name=f"kt{c}_{r}", tag=f"kt{c}_{r}")
                for r in range(3)] for c in range(NCORES)]
        # gathered V, padded per head with a ones column: [c'][ktile]
        vg = [[kvres.tile([128, 6 * 65], BF16, name=f"v{c}_{t}", tag=f"v{c}_{t}")
               for t in range(NT)] for c in range(NCORES)]
        for c in range(NCORES):
            for t in range(NT):
                nc.gpsimd.memset(vg[c][t][:, :].rearrange("p (s e) -> p s e", e=65)[:, :, 64:65], 1.0)

        wbig = [wpool.tile([128, DF], BF16, name=f"wb{d}", tag=f"wb{d}") for d in range(KD)]
        w768 = [wpool.tile([128, D], BF16, name=f"w7{i}", tag=f"w7{i}") for i in range(KD)]

        def layernorm_T(dst_bf16):
            """dst[d] <- normalize(xT) across the D (partition-tiled) axis."""
            s1 = ps_m.tile([1, PT], F32, name="s1", tag="m")
            s2 = ps_m.tile([1, PT], F32, name="s2", tag="m")
            for d in range(KD):
                xb = rot.tile([128, PT], BF16, name="xb", tag="xb")
                nc.vector.tensor_copy(out=xb[:, :], in_=xT[d][:, :])
                sq = rot.tile([128, PT], BF16, name="sq", tag="sq")
                nc.vector.tensor_mul(out=sq[:, :], in0=xb[:, :], in1=xb[:, :])
                nc.tensor.matmul(out=s1[:, :], lhsT=ones_col[:, :], rhs=xb[:, :],
                                 start=(d == 0), stop=(d == KD - 1))
                nc.tensor.matmul(out=s2[:, :], lhsT=ones_col[:, :], rhs=sq[:, :],
                                 start=(d == 0), stop=(d == KD - 1))
            mrow = rowp.tile([1, PT], F32, name="mrow", tag="row")
            nc.vector.tensor_scalar(out=mrow[:, :], in0=s1[:, :], scalar1=1.0 / D,
                                    scalar2=None, op0=ALU.mult)
            vrow = rowp.tile([1, PT], F32, name="vrow", tag="row")
            nc.vector.tensor_scalar(out=vrow[:, :], in0=s2[:, :], scalar1=1.0 / D,
                                    scalar2=None, op0=ALU.mult)
            msq = rowp.tile([1, PT], F32, name="msq", tag="row")
            nc.vector.tensor_mul(out=msq[:, :], in0=mrow[:, :], in1=mrow[:, :])
            nc.vector.tensor_sub(out=vrow[:, :], in0=vrow[:, :], in1=msq[:, :])
            srow = rowp.tile([1, PT], F32, name="srow", tag="row")
            nc.scalar.activation(out=srow[:, :], in_=vrow[:, :], func=AF.Sqrt,
                                 bias=eps_t[:, :])
            rrow = rowp.tile([1, PT], F32, name="rrow", tag="row")
            nc.vector.reciprocal(out=rrow[:, :], in_=srow[:, :])
            mr = rowp.tile([1, PT], F32, name="mr", tag="row")
            nc.vector.tensor_mul(out=mr[:, :], in0=mrow[:, :], in1=rrow[:, :])
            # broadcast [1, PT] rows across 128 partitions via K=1 matmul
            bc_r = ps_m.tile([128, PT], F32, name="bc_r", tag="m")
            nc.tensor.matmul(out=bc_r[:, :], lhsT=ones_row[:, :], rhs=rrow[:, :],
                             start=True, stop=True)
            bc_mr = ps_m.tile([128, PT], F32, name="bc_mr", tag="m")
            nc.tensor.matmul(out=bc_mr[:, :], lhsT=ones_row[:, :], rhs=mr[:, :],
                             start=True, stop=True)
            for d in range(KD):
                t32 = rot.tile([128, PT], F32, name="t32", tag="t32")
                nc.vector.tensor_mul(out=t32[:, :], in0=xT[d][:, :], in1=bc_r[:, :])
                nc.vector.tensor_sub(out=dst_bf16[d][:, :], in0=t32[:, :], in1=bc_mr[:, :])

        # ================= embedding =================
        # pos arrives pre-transposed; add it after the on-chip transpose so the
        # join is PE+one-DMA only (sync-wait slot limits).
        # aliased into the weight-slab slots (same tags) -- embed finishes
        # before the first qkv weight DMA needs them
        posv = [wpool.tile([128, PT], F32, name=f"posv{d}", tag=f"wb{d}")
                for d in range(KD)]
        for d in range(KD):
            nc.sync.dma_start(out=posv[d][:, :], in_=posT[d * 128:(d + 1) * 128, :])
        for tt in range(NT):
            xg = rot.tile([128, D], F32, name="xg", tag="xg", bufs=2)
            nc.gpsimd.indirect_dma_start(
                out=xg[:, :], out_offset=None, in_=toke[:, :],
                in_offset=bass.IndirectOffsetOnAxis(ap=idx_sb[:, tt:tt + 1], axis=0))
            for d in range(KD):
                tp = ps_s.tile([128, 128], F32, name="tp", tag="s")
                nc.tensor.transpose(out=tp[:, :], in_=xg[:, d * 128:(d + 1) * 128],
                                    identity=ident[:, :])
                nc.vector.tensor_tensor(
                    out=xT[d][:, tt * 128:(tt + 1) * 128], in0=tp[:, :],
                    in1=posv[d][:, tt * 128:(tt + 1) * 128], op=ALU.add)

        # ================= layers =================
        for l in range(L):
            last = l == L - 1
            def _stop(tag):
                return last and stop_at == tag
            # ---- LN1 -> hT ----
            layernorm_T(hT)
            if _stop("ln1"):
                return nc

            bq = biasp.tile([128, 12], F32, name="bq", tag="bq")
            nc.sync.dma_start(out=bq[:, :], in_=bqkv[l][:, :])
            bv = biasp.tile([1, D], F32, name="bv", tag="bv")
            nc.sync.dma_start(out=bv[:, :], in_=bqv[l][:, :])

            # ---- qkv: Q^T, K^T (transposed out), V (natural out) ----
            for d in range(KD):
                nc.sync.dma_start(out=wbig[d][:, :D3], in_=wqkv[l][d * 128:(d + 1) * 128, :])
            kv_in = dram.tile([2 * D, PT], BF16, name="kv_in", tag="kv_in")
            for ot in range(12):  # 0..5 Q^T, 6..11 K^T
                ps = ps_s.tile([128, PT], F32, name="ps_qk", tag="s")
                for d in range(KD):
                    nc.tensor.matmul(out=ps[:, :], lhsT=wbig[d][:, ot * 128:(ot + 1) * 128],
                                     rhs=hT[d][:, :], start=(d == 0), stop=(d == KD - 1))
                if ot < KD:
                    nc.vector.tensor_scalar(out=qT[ot][:, :], in0=ps[:, :],
                                            scalar1=bq[:, ot:ot + 1], scalar2=None, op0=ALU.add)
                else:
                    klo = rot.tile([128, PT], BF16, name="klo", tag="klo", bufs=3)
                    nc.vector.tensor_scalar(out=klo[:, :], in0=ps[:, :],
                                            scalar1=bq[:, ot:ot + 1], scalar2=None, op0=ALU.add)
                    r = ot - KD
                    nc.sync.dma_start(out=kv_in[r * 128:(r + 1) * 128, :], in_=klo[:, :])
            kv_flat = kv_in[:, :].rearrange("r c -> (r c)")
            # broadcast the V bias [1, D] across partitions once per layer
            bvb = rot.tile([128, D], F32, name="bvb", tag="bvb")
            for vh in range(2):
                bcv = ps_m.tile([128, 384], F32, name="bcv", tag="m")
                nc.tensor.matmul(out=bcv[:, :], lhsT=ones_row[:, :],
                                 rhs=bv[:, vh * 384:(vh + 1) * 384], start=True, stop=True)
                nc.vector.tensor_copy(out=bvb[:, vh * 384:(vh + 1) * 384], in_=bcv[:, :])
            for tt in range(NT):
                vloc = rot.tile([128, D], BF16, name="vloc", tag="vloc", bufs=3)
                for vh in range(2):
                    ps = ps_y.tile([128, 384], F32, name="ps_v", tag="y")
                    for d in range(KD):
                        nc.tensor.matmul(
                            out=ps[:, :],
                            lhsT=hT[d][:, tt * 128:(tt + 1) * 128],
                            rhs=wbig[d][:, D3 - D + vh * 384: D3 - D + (vh + 1) * 384],
                            start=(d == 0), stop=(d == KD - 1))
                    nc.vector.tensor_add(out=vloc[:, vh * 384:(vh + 1) * 384],
                                         in0=ps[:, :], in1=bvb[:, vh * 384:(vh + 1) * 384])
                nc.sync.dma_start(
                    out=kv_flat[D * PT + tt * 128 * D: D * PT + (tt + 1) * 128 * D]
                    .rearrange("(p e) -> p e", p=128),
                    in_=vloc[:, :])
            if _stop("qkv"):
                return nc

            # ---- AllGather K^T,V across all 8 cores ----
            kv_out = dram.tile([NCORES * 2 * D, PT], BF16, name="kv_out", tag="kv_out", addr_space="Shared")
            nc.gpsimd.collective_compute(
                "AllGather", ALU.bypass,
                replica_groups=[list(range(NCORES))],
                ins=[kv_in[:, :].opt()], outs=[kv_out[:, :].opt()])
            kvo_flat = kv_out[:, :].rearrange("r c -> (r c)")
            if _stop("ag"):
                return nc

            # ---- attention, half the heads at a time ----
            for half in range(2):
                for c in range(NCORES):
                    for r in range(3):
                        nc.sync.dma_start(
                            out=ktg[c][r][:, :],
                            in_=kv_out[c * 2 * D + half * 384 + r * 128:
                                       c * 2 * D + half * 384 + (r + 1) * 128, :])
                    for t in range(NT):
                        src = (kvo_flat[(c * 2 * D + D) * PT + t * 128 * D:
                                        (c * 2 * D + D) * PT + (t + 1) * 128 * D]
                               .rearrange("(p s e) -> p s e", p=128, e=64))
                        nc.sync.dma_start(
                            out=vg[c][t][:, :].rearrange("p (s e) -> p s e", e=65)[:, :, 0:64],
                            in_=src[:, half * 6:(half + 1) * 6, :])
                for h in range(half * 6, half * 6 + 6):
                    hs = h - half * 6
                    kr = (h * 64 - half * 384) // 128
                    kp = (h * 64) % 128
                    qtile = qT[h // 2]
                    qp = (h % 2) * 64
                    y_ps = [ps_y.tile([65, nb * 128], F32, name=f"y_ps{b}", tag="y")
                            for b in range(2)]
                    for c in range(NCORES):
                        for b in range(2):
                            for j in range(nb):
                                N = (nb - j) * 128
                                col0 = (b * nb + j) * 128
                                s_ps = ps_s.tile([128, N], F32, name="s_ps", tag="s")
                                nc.tensor.matmul(
                                    out=s_ps[:, :],
                                    lhsT=ktg[c][kr][kp:kp + 64, col0:col0 + 128],
                                    rhs=qtile[qp:qp + 64, col0:(b + 1) * nb * 128],
                                    start=True, stop=True)
                                es = esp.tile([128, N], BF16, name="es", tag="es")
                                nc.scalar.activation(out=es[:, :], in_=s_ps[:, :],
                                                     func=AF.Exp, bias=zero_col[:, :],
                                                     scale=0.125)
                                # masked diagonal block goes out-of-place so each
                                # att@V matmul depends on a single compute engine
                                esm = esp.tile([128, 128], BF16, name="esm", tag="esm")
                                nc.vector.tensor_mul(
                                    out=esm[:, :], in0=es[:, 0:128],
                                    in1=mask_sb[:, c * 128:(c + 1) * 128])
                                vh_ap = vg[c][b * nb + j][:, hs * 65:(hs + 1) * 65]
                                first = c == 0 and j == 0
                                last = c == NCORES - 1 and j == nb - 1
                                nc.tensor.matmul(
                                    out=y_ps[b][:, j * 128:(j + 1) * 128],
                                    lhsT=vh_ap, rhs=esm[:, :],
                                    start=first, stop=last and N == 128)
                                if N > 128:
                                    nc.tensor.matmul(
                                        out=y_ps[b][:, (j + 1) * 128:],
                                        lhsT=vh_ap, rhs=es[:, 128:],
                                        start=False, stop=last)
                    for b in range(2):
                        zrec = rowp.tile([1, nb * 128], F32, name="zrec", tag="row")
                        nc.vector.reciprocal(out=zrec[:, :], in_=y_ps[b][64:65, :])
                        bc = ps_m.tile([64, nb * 128], F32, name="bc", tag="m")
                        nc.tensor.matmul(out=bc[:, :], lhsT=ones_row[:, 0:64],
                                         rhs=zrec[:, :], start=True, stop=True)
                        bcs = rot.tile([64, nb * 128], F32, name="bcs", tag="bcs")
                        nc.vector.tensor_copy(out=bcs[:, :], in_=bc[:, :])
                        nc.vector.tensor_tensor(
                            out=yT[h // 2][qp:qp + 64, b * nb * 128:(b + 1) * nb * 128],
                            in0=y_ps[b][0:64, :], in1=bcs[:, :], op=ALU.mult)

            if _stop("attn"):
                return nc
            # ---- out_proj + residual: xT += Wout^T y^T ----
            for k in range(KD):
                nc.sync.dma_start(out=w768[k][:, :], in_=wout[l][k * 128:(k + 1) * 128, :])
            oacc = [ (ps_y if o < 4 else ps_m).tile([128, PT], F32, name=f"oacc{o}",
                                                    tag=("y" if o < 4 else "m"))
                     for o in range(KD)]
            for k in range(KD):
                for o in range(KD):
                    nc.tensor.matmul(out=oacc[o][:, :], lhsT=w768[k][:, o * 128:(o + 1) * 128],
                                     rhs=yT[k][:, :], start=(k == 0), stop=(k == KD - 1))
            for o in range(KD):
                nc.vector.tensor_add(out=xT[o][:, :], in0=xT[o][:, :], in1=oacc[o][:, :])
            if _stop("proj"):
                return nc

            # ---- LN2 -> hT ----
            layernorm_T(hT)

            # ---- FF: g^T tile-by-tile, immediately consumed into ff2 accumulators ----
            bft = biasp.tile([128, 24], F32, name="bft", tag="bft")
            nc.sync.dma_start(out=bft[:, :], in_=b1[l][:, :])
            for d in range(KD):
                nc.sync.dma_start(out=wbig[d][:, :], in_=w1[l][d * 128:(d + 1) * 128, :])
            facc = [ (ps_y if o < 4 else ps_m).tile([128, PT], F32, name=f"facc{o}",
                                                    tag=("y" if o < 4 else "m"))
                     for o in range(KD)]
            for ot in range(24):
                ps = ps_s.tile([128, PT], F32, name="ps_f1", tag="s")
                for d in range(KD):
                    nc.tensor.matmul(out=ps[:, :], lhsT=wbig[d][:, ot * 128:(ot + 1) * 128],
                                     rhs=hT[d][:, :], start=(d == 0), stop=(d == KD - 1))
                g = gp.tile([128, PT], BF16, name="g", tag="g")
                nc.scalar.activation(out=g[:, :], in_=ps[:, :], func=AF.Gelu,
                                     bias=bft[:, ot:ot + 1], scale=1.0)
                wslab = wpool.tile([128, D], BF16, name="w2s", tag="w2s", bufs=4)
                nc.sync.dma_start(out=wslab[:, :], in_=w2[l][ot * 128:(ot + 1) * 128, :])
                for o in range(KD):
                    nc.tensor.matmul(out=facc[o][:, :], lhsT=wslab[:, o * 128:(o + 1) * 128],
                                     rhs=g[:, :], start=(ot == 0), stop=(ot == 23))
            for o in range(KD):
                nc.vector.tensor_add(out=xT[o][:, :], in0=xT[o][:, :], in1=facc[o][:, :])

        # ================= final LN + lm_head =================
        layernorm_T(hT)
        for vc in range(NVC):
            nv = min(512, V - vc * 512)
            esl = [embp.tile([128, 512], BF16, name=f"esl{d}", tag=f"em{d}") for d in range(KD)]
            for d in range(KD):
                nc.sync.dma_start(out=esl[d][:, 0:nv],
                                  in_=embT[d * 128:(d + 1) * 128, vc * 512:vc * 512 + nv])
            for tt in range(NT):
                ps = ps_s.tile([128, 512], F32, name="ps_lm", tag="s")
                for d in range(KD):
                    nc.tensor.matmul(out=ps[:, 0:nv],
                                     lhsT=hT[d][:, tt * 128:(tt + 1) * 128],
                                     rhs=esl[d][:, 0:nv], start=(d == 0), stop=(d == KD - 1))
                lsb = logp.tile([128, 512], F32, name="lsb", tag="lsb")
                nc.vector.tensor_copy(out=lsb[:, 0:nv], in_=ps[:, 0:nv])
                nc.sync.dma_start(out=logits[tt * 128:(tt + 1) * 128, vc * 512:vc * 512 + nv],
                                  in_=lsb[:, 0:nv])
    nc.finalize()
    return nc


# ------------------------------------------------------------------
# host side
# ------------------------------------------------------------------

def _prep_inputs(nb, L, V, idx, tok_emb, pos_emb, ln1_w, ln1_b, qkv_w, out_w,
                 ln2_w, ln2_b, ff1_w, ff2_w, lnf_w, lnf_b):
    NT = 2 * nb
    PT = NT * 128
    T = 8 * nb * 128
    idx = np.asarray(idx).astype(np.int32)
    f = np.asarray

    shared = {
        "toke": f(tok_emb, dtype=np.float32),
        "embT": np.ascontiguousarray((f(tok_emb, dtype=np.float32) * f(lnf_w, dtype=np.float32)[None, :]).T).astype(bf16),
    }
    for l in range(L):
        wq = f(qkv_w[l], dtype=np.float32) * f(ln1_w[l], dtype=np.float32)[:, None]
        bq_full = f(ln1_b[l], dtype=np.float32) @ f(qkv_w[l], dtype=np.float32)  # [3D]
        shared[f"wqkv{l}"] = wq.astype(bf16)
        shared[f"bqkv{l}"] = np.ascontiguousarray(bq_full[:12 * 128].reshape(12, 128).T).astype(np.float32)
        shared[f"bqv{l}"] = bq_full[2 * D:].reshape(1, D).astype(np.float32)
        shared[f"wout{l}"] = f(out_w[l], dtype=np.float32).astype(bf16)
        w1e = f(ff1_w[l], dtype=np.float32) * f(ln2_w[l], dtype=np.float32)[:, None]
        b1_full = f(ln2_b[l], dtype=np.float32) @ f(ff1_w[l], dtype=np.float32)  # [4D]
        shared[f"w1_{l}"] = w1e.astype(bf16)
        shared[f"b1_{l}"] = np.ascontiguousarray(b1_full.reshape(24, 128).T).astype(np.float32)
        shared[f"w2_{l}"] = f(ff2_w[l], dtype=np.float32).astype(bf16)

    pos_f = f(pos_emb, dtype=np.float32)
    in_maps = []
    for c in range(NCORES):
        m = dict(shared)
        L_loc = np.arange(PT)
        b_loc = L_loc // (nb * 128)
        t_loc = 8 * (L_loc % (nb * 128)) + c
        idx_core = idx[b_loc, t_loc]  # [PT]
        m["idxs"] = np.ascontiguousarray(idx_core.reshape(NT, 128).T).astype(np.int32)
        m["posT"] = np.ascontiguousarray(pos_f[t_loc].T).astype(np.float32)
        mk = np.zeros((128, NCORES * 128), dtype=np.float32)
        for cp in range(NCORES):
            mk[:, cp * 128:(cp + 1) * 128] = np.triu(np.ones((128, 128), np.float32),
                                                     0 if cp <= c else 1)
        m["masks"] = mk.astype(bf16)
        in_maps.append(m)
    return in_maps


_NC_CACHE = {}


def _get_nc(nb, L, V):
    key = (nb, L, V)
    if key not in _NC_CACHE:
        _NC_CACHE[key] = build_nc(nb, L, V)
    return _NC_CACHE[key]


def run_on_hw(nb, L, V, inputs, trace=False):
    from concourse import bass_utils
    nc = _get_nc(nb, L, V)
    in_maps = _prep_inputs(nb, L, V, **inputs)
    res = bass_utils.run_bass_kernel_spmd(nc, in_maps, core_ids=list(range(NCORES)),
                                          trace=trace)
    return res


def assemble(nb, L, V, results, lnf_b, tok_emb):
    NT = 2 * nb
    PT = NT * 128
    T = 8 * nb * 128
    out = np.empty((2, T, V), dtype=np.float32)
    for c in range(NCORES):
        lg = results[c]["logits"].reshape(2, nb * 128, V)
        out[:, c::8, :] = lg
    lnf_b = np.asarray(lnf_b, dtype=np.float32)
    if np.any(lnf_b):
        out += (lnf_b @ np.asarray(tok_emb, dtype=np.float32).T)[None, None, :]
    return out


def kernel(**inputs):
    nb, L, V = 2, 6, 32000
    res = run_on_hw(nb, L, V, inputs)
    return assemble(nb, L, V, res.results, inputs["lnf_b"], inputs["tok_emb"])



# revision 34
# speedup vs baseline: 1.3611x; 1.3611x over previous
"""GPT-style dense transformer on 8 Trainium2 NeuronCores (v2).

Sharding: token-parallel. Core c owns positions t = 8*i + c of BOTH batches.
Local token layout is b-major: tile tt = 2*b + q (q = local 128-token tile),
so per-batch query columns are contiguous [256b, 256b+256).

All per-token work (LN, qkv, out_proj, ff, lm_head) is local; attention
needs all keys, so K^T and V are AllGathered in fp8 (two collectives per
layer, issued as soon as K resp. V are produced, overlapping the Q compute
and the head-pair pipeline start).

The residual stream lives transposed in SBUF as x^T [D, 512] f32. Every
matmul consumes natural-layout weights as the stationary operand and
transposed bf16 activations as the moving operand.

Attention is processed in head PAIRS (heads 2r, 2r+1 share K^T/Q^T tile r,
partitions 0:64 / 64:128): score matmuls of the two heads use disjoint PE
row groups and run concurrently. Scores for one (source core, batch) pair
of heads are packed into a 3-PSUM-bank tile with no padding and exp'd by a
single ACT instruction [128, 1536] -> bf16 SBUF; causal masks are applied
post-exp with two-region strided DVE multiplies. exp'd scores flow into
att@V with a 65th ones-column on V accumulating the softmax denominator.
"""

import os
import sys

for _p in ("/opt/trn_rl_repo",):
    if _p not in sys.path:
        sys.path.insert(0, _p)

import numpy as np
import ml_dtypes

import concourse.bass as bass
import concourse.bacc as bacc
import concourse.mybir as mybir
import concourse.tile as tile
from concourse.masks import make_identity

BF16 = mybir.dt.bfloat16
F32 = mybir.dt.float32
FP8 = mybir.dt.float8e4
I32 = mybir.dt.int32
AF = mybir.ActivationFunctionType
ALU = mybir.AluOpType

NCORES = 8
H = 12
HD = 64
D = 768
D3 = 3 * D       # 2304
DF = 4 * D       # 3072
KD = D // 128    # 6 d-tiles
NPAIR = 6        # head pairs
PT = 512         # tokens per core
NT = 4           # 128-token tiles per core; tt = 2*b + q
EPS = 1e-5
VCHUNK = 512

bf16 = ml_dtypes.bfloat16

# scores batch layout: one (source-core, head-pair) -> [128, 1536] f32 (3 banks).
# entries: (head_sel, jk, off, n); head_sel 0=A(partitions 0:64) 1=B(64:128).
# Emission alternates A/B so the K=64 matmuls run in concurrent PE row groups.
# Every region stays inside one PSUM bank (bank boundaries at 512/1024).
SCORE_SLOTS = {
    0: [(0, 0, 0, 256), (1, 1, 384, 128), (0, 1, 256, 128), (1, 0, 512, 256)],
    1: [(0, 0, 768, 256), (1, 1, 1152, 128), (0, 1, 1024, 128), (1, 0, 1280, 256)],
}
# diag (causally masked) regions per batch: (off0, stride, nblk) -> nblk blocks
# of 128 cols at off0, off0+stride, ...
MASK_VIEWS = {
    0: [(0, 256, 2), (384, 128, 2)],
    1: [(768, 256, 2), (1152, 128, 2)],
}
# per head_sel: which (b, slot index) carries start/stop of the y accumulation
Y_FIRST = {0: (0, 0), 1: (0, 1)}   # (b, index in SCORE_SLOTS[b])
Y_LAST = {0: (1, 2), 1: (1, 3)}


PHASES = ["ln1", "kag", "vag", "q", "attn_g", "attn_s", "attn_e", "attn_m", "attn_v", "attn", "proj", "ff"]


def build_nc(nb, L, V, stop_at=None):
    assert nb == 2, "v2 kernel is hardcoded for T=2048 (nb=2)"
    plim = PHASES.index(stop_at) if stop_at else len(PHASES) - 1
    def enabled(ph):
        return PHASES.index(ph) <= plim
    NVC = (V + VCHUNK - 1) // VCHUNK

    nc = bacc.Bacc("TRN2", target_bir_lowering=False, num_devices=NCORES)

    # ---- I/O ----
    idxs = nc.dram_tensor("idxs", [128, NT], I32, kind="ExternalInput")
    posT = nc.dram_tensor("posT", [D, PT], F32, kind="ExternalInput")
    masks = nc.dram_tensor("masks", [128, NCORES * 256], BF16, kind="ExternalInput")
    selm = nc.dram_tensor("selm", [128, 256], F32, kind="ExternalInput")
    toke = nc.dram_tensor("toke", [V, D], F32, kind="ExternalInput")
    embT = nc.dram_tensor("embT", [D, V], BF16, kind="ExternalInput")
    wqkv = [nc.dram_tensor(f"wqkv{l}", [128, KD * D3], BF16, kind="ExternalInput") for l in range(L)]
    bqkv = [nc.dram_tensor(f"bqkv{l}", [128, 12], F32, kind="ExternalInput") for l in range(L)]
    bqv = [nc.dram_tensor(f"bqv{l}", [1, D], F32, kind="ExternalInput") for l in range(L)]
    wout = [nc.dram_tensor(f"wout{l}", [128, KD * D], BF16, kind="ExternalInput") for l in range(L)]
    w1 = [nc.dram_tensor(f"w1_{l}", [128, KD * DF], BF16, kind="ExternalInput") for l in range(L)]
    b1 = [nc.dram_tensor(f"b1_{l}", [128, 24], F32, kind="ExternalInput") for l in range(L)]
    w2 = [nc.dram_tensor(f"w2_{l}", [128, 24 * D], BF16, kind="ExternalInput") for l in range(L)]
    logits = nc.dram_tensor("logits", [PT, V], BF16, kind="ExternalOutput")
    DBG = bool(os.environ.get("KV_DEBUG"))
    if DBG:
        dbg_q = nc.dram_tensor("dbg_q", [64, 2 * PT], BF16, kind="ExternalOutput")
        dbg_k = nc.dram_tensor("dbg_k", [64, 2 * PT], BF16, kind="ExternalOutput")
        dbg_es = nc.dram_tensor("dbg_es", [128, 1536], BF16, kind="ExternalOutput")
        dbg_y = nc.dram_tensor("dbg_y", [130, PT], BF16, kind="ExternalOutput")
        dbg_h = nc.dram_tensor("dbg_h", [128, PT], BF16, kind="ExternalOutput")
        dbg_yt = nc.dram_tensor("dbg_yt", [128, PT], BF16, kind="ExternalOutput")
        dbg_xp = nc.dram_tensor("dbg_xp", [128, PT], BF16, kind="ExternalOutput")
        dbg_xf = nc.dram_tensor("dbg_xf", [128, PT], BF16, kind="ExternalOutput")

    from contextlib import ExitStack
    with tile.TileContext(nc) as tc, ExitStack() as ctx:
        def pool(**kw):
            return ctx.enter_context(tc.tile_pool(**kw))

        const = pool(name="const", bufs=1)
        resid = pool(name="resid", bufs=1)
        acts = pool(name="acts", bufs=1)
        kpool = pool(name="kpool", bufs=2)
        vstr = pool(name="vstr", bufs=2)
        wq_p = pool(name="wq_p", bufs=1)
        wo_p = pool(name="wo_p", bufs=1)
        wf1_p = pool(name="wf1_p", bufs=1)
        wf2_p = pool(name="wf2_p", bufs=2)
        biasp = pool(name="biasp", bufs=2)
        stage = pool(name="stage", bufs=2)
        esp = pool(name="esp", bufs=1 if os.environ.get("KV_DEBUG") else 2)
        rows = pool(name="rows", bufs=6)
        rot = pool(name="rot", bufs=2)
        gp = pool(name="gp", bufs=2)
        logp = pool(name="logp", bufs=2)
        ps3 = pool(name="ps3", bufs=2, space="PSUM")   # 2 x 3 banks
        ps1 = pool(name="ps1", bufs=2, space="PSUM")   # 2 x 1 bank
        dram = pool(name="dram", bufs=2, space="DRAM")

        # ---- constants ----
        ident = const.tile([128, 128], F32, name="ident", tag="ident")
        make_identity(nc, ident)
        ones_col = const.tile([128, 1], F32, name="ones_col", tag="ones_col")
        nc.gpsimd.memset(ones_col[:, :], 1.0)
        ones_row = const.tile([1, 128], F32, name="ones_row", tag="ones_row")
        nc.gpsimd.memset(ones_row[:, :], 1.0)
        ones16 = const.tile([128, 12], BF16, name="ones16", tag="ones16")
        nc.gpsimd.memset(ones16[:, :], 1.0)
        sel_sb = const.tile([128, 256], F32, name="sel_sb", tag="sel_sb")
        nc.sync.dma_start(out=sel_sb[:, :], in_=selm[:, :])
        eps_t = const.tile([1, 1], F32, name="eps_t", tag="eps_t")
        nc.gpsimd.memset(eps_t[:, :], EPS)
        mask_sb = const.tile([128, NCORES * 256], BF16, name="mask_sb", tag="mask_sb")
        nc.sync.dma_start(out=mask_sb[:, :], in_=masks[:, :])
        idx_sb = const.tile([128, NT], I32, name="idx_sb", tag="idx_sb")
        nc.sync.dma_start(out=idx_sb[:, :], in_=idxs[:, :])

        # ---- persistent state ----
        xT = [resid.tile([128, PT], F32, name=f"xt{d}", tag=f"xt{d}") for d in range(KD)]
        hT = [acts.tile([128, PT], BF16, name=f"ht{d}", tag=f"ht{d}") for d in range(KD)]
        qT = [acts.tile([64, 2 * PT], FP8, name=f"qt{d}", tag=f"qt{d}") for d in range(KD)]
        yT = [acts.tile([128, PT], BF16, name=f"yt{d}", tag=f"yt{d}") for d in range(KD)]
        wq_sb = wq_p.tile([128, KD * D3], BF16, name="wq_sb", tag="wq_sb")
        wo_sb = wo_p.tile([128, KD * D], BF16, name="wo_sb", tag="wo_sb")
        w1_sb = wf1_p.tile([128, KD * DF], BF16, name="w1_sb", tag="w1_sb")

        def layernorm_T(dst):
            """dst[d] (bf16) <- normalize(xT) across the partition-tiled D axis."""
            s1 = ps1.tile([1, PT], F32, name="s1", tag="b1")
            s2 = ps1.tile([1, PT], F32, name="s2", tag="b1")
            for d in range(KD):
                sq = rot.tile([128, PT], F32, name="sq", tag="sq")
                nc.vector.tensor_mul(out=sq[:, :], in0=xT[d][:, :], in1=xT[d][:, :])
                nc.tensor.matmul(out=s1[:, :], lhsT=ones_col[:, :], rhs=xT[d][:, :],
                                 start=(d == 0), stop=(d == KD - 1))
                nc.tensor.matmul(out=s2[:, :], lhsT=ones_col[:, :], rhs=sq[:, :],
                                 start=(d == 0), stop=(d == KD - 1))
            mrow = rows.tile([1, PT], F32, name="mrow", tag="row")
            nc.vector.tensor_scalar(out=mrow[:, :], in0=s1[:, :], scalar1=1.0 / D,
                                    scalar2=None, op0=ALU.mult)
            msq = rows.tile([1, PT], F32, name="msq", tag="row")
            nc.scalar.activation(out=msq[:, :], in_=s1[:, :], func=AF.Square,
                                 scale=1.0 / D)
            vrow = rows.tile([1, PT], F32, name="vrow", tag="row")
            nc.vector.tensor_scalar(out=vrow[:, :], in0=s2[:, :], scalar1=1.0 / D,
                                    scalar2=None, op0=ALU.mult)
            nc.vector.tensor_sub(out=vrow[:, :], in0=vrow[:, :], in1=msq[:, :])
            srow = rows.tile([1, PT], F32, name="srow", tag="row")
            nc.scalar.activation(out=srow[:, :], in_=vrow[:, :], func=AF.Ln,
                                 bias=eps_t[:, :])
            rrow = rows.tile([1, PT], F32, name="rrow", tag="row")
            nc.scalar.activation(out=rrow[:, :], in_=srow[:, :], func=AF.Exp,
                                 scale=-0.5)
            mr = rows.tile([1, PT], F32, name="mr", tag="row")
            nc.vector.tensor_mul(out=mr[:, :], in0=mrow[:, :], in1=rrow[:, :])
            bc_r = ps1.tile([128, PT], F32, name="bc_r", tag="b1")
            nc.tensor.matmul(out=bc_r[:, :], lhsT=ones_row[:, :], rhs=rrow[:, :],
                             start=True, stop=True)
            bc_mr = ps1.tile([128, PT], F32, name="bc_mr", tag="b1")
            nc.tensor.matmul(out=bc_mr[:, :], lhsT=ones_row[:, :], rhs=mr[:, :],
                             start=True, stop=True)
            for d in range(KD):
                t32 = rot.tile([128, PT], F32, name="t32", tag="t32", bufs=1)
                nc.vector.tensor_mul(out=t32[:, :], in0=xT[d][:, :], in1=bc_r[:, :])
                nc.vector.tensor_sub(out=dst[d][:, :], in0=t32[:, :], in1=bc_mr[:, :])
                if d % 2 == 1:
                    nc.tensor.ldweights(weights=dst[d][:, 0:128])

        # ================= embedding =================
        for d in range(KD):
            nc.sync.dma_start(out=xT[d][:, :], in_=posT[d * 128:(d + 1) * 128, :])
        for tt in range(NT):
            xg = rot.tile([128, D], F32, name="xg", tag="xg", bufs=1)
            nc.gpsimd.indirect_dma_start(
                out=xg[:, :], out_offset=None, in_=toke[:, :],
                in_offset=bass.IndirectOffsetOnAxis(ap=idx_sb[:, tt:tt + 1], axis=0))
            for d in range(KD):
                tp = ps1.tile([128, 128], F32, name="tp", tag="b1")
                nc.tensor.transpose(out=tp[:, :], in_=xg[:, d * 128:(d + 1) * 128],
                                    identity=ident[:, :])
                nc.vector.tensor_tensor(
                    out=xT[d][:, tt * 128:(tt + 1) * 128], in0=tp[:, :],
                    in1=xT[d][:, tt * 128:(tt + 1) * 128], op=ALU.add)

        # ================= layers =================
        for l in range(L):
            nc.sync.dma_start(out=wq_sb[:, :], in_=wqkv[l][:, :])
            bq = biasp.tile([128, 12], F32, name="bq", tag="bq")
            nc.sync.dma_start(out=bq[:, :], in_=bqkv[l][:, :])
            bv = biasp.tile([1, D], F32, name="bv", tag="bv")
            nc.sync.dma_start(out=bv[:, :], in_=bqv[l][:, :])

            # ---- LN1 -> hT ----
            layernorm_T(hT)
            if not enabled("kag"):
                continue

            # ---- K^T -> kva_in, AllGather ----
            kva_in = dram.tile([D, PT], FP8, name="kva_in", tag="kva_in")
            for r in range(KD):
                ps = ps1.tile([128, PT], F32, name="ps_k", tag="b1")
                for d in range(KD):
                    nc.tensor.matmul(out=ps[:, :],
                                     lhsT=wq_sb[:, d * D3 + (KD + r) * 128: d * D3 + (KD + r) * 128 + 128],
                                     rhs=hT[d][:, :], start=(d == 0), stop=(d == KD - 1))
                klo = stage.tile([128, PT], FP8, name="klo", tag="klo")
                nc.vector.tensor_scalar(out=klo[:, :], in0=ps[:, :],
                                        scalar1=bq[:, KD + r:KD + r + 1], scalar2=None,
                                        op0=ALU.add)
                nc.sync.dma_start(out=kva_in[r * 128:(r + 1) * 128, :], in_=klo[:, :])
            kva_out = dram.tile([NCORES * D, PT], FP8, name="kva_out", tag="kva_out",
                                addr_space="Shared")
            nc.gpsimd.collective_compute(
                "AllGather", ALU.bypass,
                replica_groups=[list(range(NCORES))],
                ins=[kva_in[:, :].opt()], outs=[kva_out[:, :].opt()])

            if not enabled("vag"):
                continue
            # ---- V (padded, pair-major) -> kvb_in, AllGather ----
            bvb = rot.tile([128, D], F32, name="bvb", tag="bvb", bufs=1)
            for vh in range(2):
                bcv = ps3.tile([128, 384], F32, name="bcv", tag="b3")
                nc.tensor.matmul(out=bcv[:, :], lhsT=ones_row[:, :],
                                 rhs=bv[:, vh * 384:(vh + 1) * 384], start=True, stop=True)
                nc.vector.tensor_copy(out=bvb[:, vh * 384:(vh + 1) * 384], in_=bcv[:, :])
            kvb_in = dram.tile([NPAIR * NT * 128, 130], BF16, name="kvb_in", tag="kvb_in")
            for tt in range(NT):
                vloc = stage.tile([128, 780], BF16, name="vloc", tag="vloc")
                nc.vector.tensor_copy(
                    out=vloc[:, :].rearrange("p (s e) -> p s e", e=65)[:, :, 64:65],
                    in_=ones16[:, :].rearrange("p (s e) -> p s e", e=1))
                for vh in range(2):
                    ps = ps1.tile([128, 384], F32, name="ps_v", tag="b1")
                    for d in range(KD):
                        nc.tensor.matmul(
                            out=ps[:, :],
                            lhsT=hT[d][:, tt * 128:(tt + 1) * 128],
                            rhs=wq_sb[:, d * D3 + 2 * D + vh * 384: d * D3 + 2 * D + (vh + 1) * 384],
                            start=(d == 0), stop=(d == KD - 1))
                    nc.vector.tensor_tensor(
                        out=vloc[:, vh * 390:(vh + 1) * 390]
                        .rearrange("p (s e) -> p s e", e=65)[:, :, 0:64],
                        in0=ps[:, :].rearrange("p (s e) -> p s e", e=64),
                        in1=bvb[:, vh * 384:(vh + 1) * 384]
                        .rearrange("p (s e) -> p s e", e=64),
                        op=ALU.add)
                for r in range(NPAIR):
                    nc.sync.dma_start(
                        out=kvb_in[(r * NT + tt) * 128:(r * NT + tt + 1) * 128, :],
                        in_=vloc[:, r * 130:(r + 1) * 130])
            kvb_out = dram.tile([NCORES * NPAIR * NT * 128, 130], BF16,
                                name="kvb_out", tag="kvb_out", addr_space="Shared")
            nc.gpsimd.collective_compute(
                "AllGather", ALU.bypass,
                replica_groups=[list(range(NCORES))],
                ins=[kvb_in[:, :].opt()], outs=[kvb_out[:, :].opt()])

            if not enabled("q"):
                continue
            # ---- Q^T (overlaps the K/V AllGathers) ----
            for r in range(KD):
                ps = ps1.tile([128, PT], F32, name="ps_q", tag="b1")
                for d in range(KD):
                    nc.tensor.matmul(out=ps[:, :],
                                     lhsT=wq_sb[:, d * D3 + r * 128: d * D3 + r * 128 + 128],
                                     rhs=hT[d][:, :], start=(d == 0), stop=(d == KD - 1))
                for hs in range(2):
                    nc.vector.tensor_scalar(
                        out=qT[r][0:64, hs * PT:(hs + 1) * PT],
                        in0=ps[hs * 64:(hs + 1) * 64, :],
                        scalar1=bq[hs * 64:(hs + 1) * 64, r:r + 1],
                        scalar2=None, op0=ALU.add)
            if DBG and l == 0:
                dtq = gp.tile([64, 2 * PT], BF16, name="dtq", tag="dbg", bufs=1)
                nc.vector.tensor_copy(out=dtq[:, :], in_=qT[0][:, :])
                nc.sync.dma_start(out=dbg_q[:, :], in_=dtq[:, :])
                dth = gp.tile([128, PT], BF16, name="dth", tag="dbg", bufs=1)
                nc.vector.tensor_copy(out=dth[:, :], in_=hT[0][:, :])
                nc.sync.dma_start(out=dbg_h[:, :], in_=dth[:, :])
            if not enabled("attn_g"):
                continue

            # prefetch next-phase weights during AG/attention
            nc.sync.dma_start(out=wo_sb[:, :], in_=wout[l][:, :])
            nc.sync.dma_start(out=w1_sb[:, :], in_=w1[l][:, :])

            # ---- attention, one head pair at a time ----
            # z rows parked at 32-aligned partitions: pair r -> col group r//2,
            # head A/B at partition (2r%4)*32 / ((2r+1)%4)*32
            zb = rows.tile([128, 1536], F32, name="zb", tag="zb", bufs=1)
            nc.gpsimd.memset(zb[:, :], 1.0)
            for r in range(NPAIR):
                kp = kpool.tile([64, NCORES * 2 * PT], FP8, name="kp", tag="kp")
                vgp = vstr.tile([128, NCORES * NT * 130], BF16, name="vgp", tag="vgp")
                for c in range(NCORES):
                    nc.sync.dma_start(
                        out=kp[0:64, c * 2 * PT:(c + 1) * 2 * PT]
                        .rearrange("p (g n) -> p g n", g=2),
                        in_=kva_out[c * D + r * 128: c * D + r * 128 + 128, :]
                        .rearrange("(g p) n -> p g n", p=64))
                    nc.sync.dma_start(
                        out=vgp[:, c * NT * 130:(c + 1) * NT * 130]
                        .rearrange("p (t e) -> p t e", t=NT),
                        in_=kvb_out[(c * NPAIR + r) * NT * 128:
                                    (c * NPAIR + r + 1) * NT * 128, :]
                        .rearrange("(t p) e -> p t e", p=128))
                if DBG and l == 0 and r == 0:
                    dtk = gp.tile([64, 2 * PT], BF16, name="dtk", tag="dbg", bufs=1)
                    nc.vector.tensor_copy(out=dtk[:, :], in_=kp[0:64, 0:2 * PT])
                    nc.sync.dma_start(out=dbg_k[:, :], in_=dtk[:, :])
                if not enabled("attn_s"):
                    continue
                y_ps = ([ps1.tile([65, PT], F32, name=f"y_ps{s}", tag="b1")
                        for s in range(2)] if enabled("attn_v") else None)
                for c in range(NCORES):
                    satt = ps3.tile([128, 1536], F32, name="satt", tag="b3")
                    for b in range(2):
                        for (hs, jk, off, n) in SCORE_SLOTS[b]:
                            qlo = 256 * b + 128 * jk if n == 128 else 256 * b
                            nc.tensor.matmul(
                                out=satt[:, off:off + n],
                                lhsT=kp[0:64, c * 2 * PT + hs * PT + 256 * b + 128 * jk:
                                        c * 2 * PT + hs * PT + 256 * b + 128 * jk + 128],
                                rhs=qT[r][0:64, hs * PT + qlo:hs * PT + qlo + n],
                                start=True, stop=True)
                    if not enabled("attn_e"):
                        continue
                    es = esp.tile([128, 1536], BF16, name="es", tag="es")
                    if os.environ.get("KV_EXP_SPLIT"):
                        for b_ in range(2):
                            for (hs_, jk_, off_, n_) in SCORE_SLOTS[b_]:
                                nc.scalar.activation(out=es[:, off_:off_ + n_],
                                                     in_=satt[:, off_:off_ + n_],
                                                     func=AF.Exp, scale=0.125)
                    else:
                        nc.scalar.activation(out=es[:, :], in_=satt[:, :], func=AF.Exp,
                                             scale=0.125)
                    if not enabled("attn_m"):
                        continue
                    for b in range(2):
                        if os.environ.get("KV_NO_MASK"):
                            break
                        for (off0, stride, nblk) in MASK_VIEWS[b]:
                            if os.environ.get("KV_MASK_SIMPLE"):
                                for kb in range(nblk):
                                    nc.vector.tensor_tensor(
                                        out=es[:, off0 + kb * stride:off0 + kb * stride + 128],
                                        in0=es[:, off0 + kb * stride:off0 + kb * stride + 128],
                                        in1=mask_sb[:, c * 256:c * 256 + 128], op=ALU.mult)
                                continue
                            ev = (es[:, off0:off0 + stride * nblk]
                                  .rearrange("p (k x) -> p k x", x=stride)[:, :, 0:128])
                            mv = (mask_sb[:, c * 256:(c + 1) * 256]
                                  .rearrange("p (k x) -> p k x", x=128))
                            nc.vector.tensor_tensor(out=ev, in0=ev, in1=mv, op=ALU.mult)
                    if DBG and l == 0 and r == 0 and c == 0:
                        dtmp = gp.tile([128, 1536], BF16, name="dtmp", tag="dbg", bufs=1)
                        nc.vector.tensor_copy(out=dtmp[:, :], in_=es[:, :])
                        nc.sync.dma_start(out=dbg_es[:, :], in_=dtmp[:, :])
                    if not enabled("attn_v"):
                        continue
                    for b in range(2):
                        for si, (hs, jk, off, n) in enumerate(SCORE_SLOTS[b]):
                            tt = 2 * b + jk
                            ycol = 256 * b + 128 * jk if n == 128 else 256 * b
                            first = c == 0 and Y_FIRST[hs] == (b, si)
                            lastv = c == NCORES - 1 and Y_LAST[hs] == (b, si)
                            nc.tensor.matmul(
                                out=y_ps[hs][:, ycol:ycol + n],
                                lhsT=vgp[:, (c * NT + tt) * 130 + hs * 65:
                                         (c * NT + tt) * 130 + hs * 65 + 65],
                                rhs=es[:, off:off + n],
                                start=first, stop=lastv)
                if DBG and l == 0 and r == 0:
                    for hs in range(2):
                        dty = gp.tile([65, PT], BF16, name="dty", tag="dbg", bufs=1)
                        nc.vector.tensor_copy(out=dty[:, :], in_=y_ps[hs][:, :])
                        nc.sync.dma_start(out=dbg_y[hs * 65:(hs + 1) * 65, :], in_=dty[:, :])
                if not enabled("attn"):
                    continue
                # evacuate y (unnormalized) into yT and z into the zb park
                for hs in range(2):
                    zrow = ((2 * r + hs) % 4) * 32
                    zcol = (r // 2) * PT
                    nc.vector.tensor_copy(out=zb[zrow:zrow + 1, zcol:zcol + PT],
                                          in_=y_ps[hs][64:65, :])
                    nc.vector.tensor_copy(out=yT[r][hs * 64:(hs + 1) * 64, :],
                                          in_=y_ps[hs][0:64, :])
            if not enabled("attn"):
                continue
            # batched softmax denominators: 1/z = exp(-ln z) over the whole park
            nc.scalar.activation(out=zb[:, :], in_=zb[:, :], func=AF.Ln)
            nc.scalar.activation(out=zb[:, :], in_=zb[:, :], func=AF.Exp,
                                 scale=-1.0)
            for r in range(NPAIR):
                bcz = ps3.tile([128, 1536], F32, name="bcz", tag="b3")
                nc.tensor.matmul(out=bcz[:, 0:PT],
                                 lhsT=sel_sb[:, (r % 2) * 128:(r % 2) * 128 + 128],
                                 rhs=zb[:, (r // 2) * PT:(r // 2) * PT + PT],
                                 start=True, stop=True)
                nc.vector.tensor_mul(out=yT[r][:, :], in0=yT[r][:, :],
                                     in1=bcz[:, 0:PT])
            if DBG and l == 0:
                dyt = gp.tile([128, PT], BF16, name="dyt", tag="dbg", bufs=1)
                nc.vector.tensor_copy(out=dyt[:, :], in_=yT[0][:, :])
                nc.sync.dma_start(out=dbg_yt[:, :], in_=dyt[:, :])
            if not enabled("proj"):
                continue

            # ---- out_proj + residual ----
            oacc = [ps3.tile([128, 1536], F32, name=f"oacc{s}", tag="b3")
                    for s in range(2)]
            for r in range(NPAIR):
                for o in range(KD):
                    nc.tensor.matmul(
                        out=oacc[o // 3][:, (o % 3) * 512:(o % 3) * 512 + 512],
                        lhsT=wo_sb[:, r * D + o * 128: r * D + o * 128 + 128],
                        rhs=yT[r][:, :], start=(r == 0), stop=(r == NPAIR - 1))
            for o in range(KD):
                nc.vector.tensor_add(out=xT[o][:, :], in0=xT[o][:, :],
                                     in1=oacc[o // 3][:, (o % 3) * 512:(o % 3) * 512 + 512])
            if DBG and l == 0:
                dxp = gp.tile([128, PT], BF16, name="dxp", tag="dbg", bufs=1)
                nc.vector.tensor_copy(out=dxp[:, :], in_=xT[0][:, :])
                nc.sync.dma_start(out=dbg_xp[:, :], in_=dxp[:, :])
            if not enabled("ff"):
                continue

            # ---- LN2 -> hT ----
            layernorm_T(hT)

            # ---- FF ----
            bft = biasp.tile([128, 24], F32, name="bft", tag="bft")
            nc.sync.dma_start(out=bft[:, :], in_=b1[l][:, :])
            facc = [ps3.tile([128, 1536], F32, name=f"facc{s}", tag="b3")
                    for s in range(2)]
            for ot in range(24):
                if ot % 4 == 0:
                    wslab = wf2_p.tile([128, 4 * D], BF16, name="wslab", tag="wslab")
                    nc.sync.dma_start(out=wslab[:, :],
                                      in_=w2[l][:, (ot // 4) * 4 * D:(ot // 4 + 1) * 4 * D])
                ps = ps1.tile([128, PT], F32, name="ps_f1", tag="b1")
                for d in range(KD):
                    nc.tensor.matmul(out=ps[:, :],
                                     lhsT=w1_sb[:, d * DF + ot * 128: d * DF + ot * 128 + 128],
                                     rhs=hT[d][:, :], start=(d == 0), stop=(d == KD - 1))
                g = gp.tile([128, PT], BF16, name="g", tag="g")
                nc.scalar.activation(out=g[:, :], in_=ps[:, :], func=AF.Gelu,
                                     bias=bft[:, ot:ot + 1], scale=1.0)
                for o in range(KD):
                    nc.tensor.matmul(
                        out=facc[o // 3][:, (o % 3) * 512:(o % 3) * 512 + 512],
                        lhsT=wslab[:, (ot % 4) * D + o * 128:(ot % 4) * D + o * 128 + 128],
                        rhs=g[:, :], start=(ot == 0), stop=(ot == 23))
            for o in range(KD):
                nc.vector.tensor_add(out=xT[o][:, :], in0=xT[o][:, :],
                                     in1=facc[o // 3][:, (o % 3) * 512:(o % 3) * 512 + 512])
            if DBG and l == 0:
                dxf = gp.tile([128, PT], BF16, name="dxf", tag="dbg", bufs=1)
                nc.vector.tensor_copy(out=dxf[:, :], in_=xT[0][:, :])
                nc.sync.dma_start(out=dbg_xf[:, :], in_=dxf[:, :])

        # ================= final LN + lm_head =================
        layernorm_T(hT)
        for vc in range(NVC):
            nv = min(VCHUNK, V - vc * VCHUNK)
            esl = wf2_p.tile([128, KD * VCHUNK], BF16, name="esl", tag="wslab")
            nc.sync.dma_start(
                out=esl[:, :].rearrange("p (r v) -> p r v", v=VCHUNK)[:, :, 0:nv],
                in_=embT[:, vc * VCHUNK:vc * VCHUNK + nv]
                .rearrange("(r p) v -> p r v", p=128))
            for th in range(2):
                lsb = logp.tile([128, 2 * VCHUNK], BF16, name="lsb", tag="lsb")
                for ti in range(2):
                    tt = th * 2 + ti
                    ps = ps1.tile([128, VCHUNK], F32, name="ps_lm", tag="b1")
                    for d in range(KD):
                        nc.tensor.matmul(out=ps[:, 0:nv],
                                         lhsT=hT[d][:, tt * 128:(tt + 1) * 128],
                                         rhs=esl[:, d * VCHUNK:d * VCHUNK + nv],
                                         start=(d == 0), stop=(d == KD - 1))
                    nc.vector.tensor_copy(out=lsb[:, ti * VCHUNK:ti * VCHUNK + nv],
                                          in_=ps[:, 0:nv])
                nc.sync.dma_start(
                    out=logits[th * 256:(th + 1) * 256, vc * VCHUNK:vc * VCHUNK + nv]
                    .rearrange("(t p) v -> p t v", p=128),
                    in_=lsb[:, :].rearrange("p (t v) -> p t v", t=2)[:, :, 0:nv])
    nc.finalize()
    return nc


# ------------------------------------------------------------------
# host side
# ------------------------------------------------------------------

def _prep_inputs(nb, L, V, idx, tok_emb, pos_emb, ln1_w, ln1_b, qkv_w, out_w,
                 ln2_w, ln2_b, ff1_w, ff2_w, lnf_w, lnf_b):
    idx = np.asarray(idx).astype(np.int32)
    f = np.asarray

    def chunk128(w, inner):
        """[K*128, inner] -> [128, K*inner] with chunk k at cols [k*inner, ...)"""
        k = w.shape[0] // 128
        return np.ascontiguousarray(
            w.reshape(k, 128, inner).transpose(1, 0, 2).reshape(128, k * inner))

    shared = {
        "toke": f(tok_emb, dtype=np.float32),
        "embT": np.ascontiguousarray(
            (f(tok_emb, dtype=np.float32) * f(lnf_w, dtype=np.float32)[None, :]).T
        ).astype(bf16),
    }
    for l in range(L):
        wq = f(qkv_w[l], dtype=np.float32) * f(ln1_w[l], dtype=np.float32)[:, None]
        bq_full = f(ln1_b[l], dtype=np.float32) @ f(qkv_w[l], dtype=np.float32)
        shared[f"wqkv{l}"] = chunk128(wq, 3 * D).astype(bf16)
        shared[f"bqkv{l}"] = np.ascontiguousarray(
            bq_full[:12 * 128].reshape(12, 128).T).astype(np.float32)
        shared[f"bqv{l}"] = bq_full[2 * D:].reshape(1, D).astype(np.float32)
        shared[f"wout{l}"] = chunk128(f(out_w[l], dtype=np.float32), D).astype(bf16)
        w1e = f(ff1_w[l], dtype=np.float32) * f(ln2_w[l], dtype=np.float32)[:, None]
        b1_full = f(ln2_b[l], dtype=np.float32) @ f(ff1_w[l], dtype=np.float32)
        shared[f"w1_{l}"] = chunk128(w1e, DF).astype(bf16)
        shared[f"b1_{l}"] = np.ascontiguousarray(
            b1_full.reshape(24, 128).T).astype(np.float32)
        shared[f"w2_{l}"] = chunk128(f(ff2_w[l], dtype=np.float32), D).astype(bf16)

    sel = np.zeros((128, 256), dtype=np.float32)
    for par in range(2):
        for m_ in range(128):
            sel[(2 * par + m_ // 64) * 32, par * 128 + m_] = 1.0
    shared["selm"] = sel

    pos_f = f(pos_emb, dtype=np.float32)
    in_maps = []
    for c in range(NCORES):
        m = dict(shared)
        # local col = tt*128 + p ; tt = 2*b + q ; global (b, 8*(q*128+p)+c)
        cols = np.arange(PT)
        tt = cols // 128
        p = cols % 128
        b_loc = tt // 2
        q_loc = tt % 2
        t_loc = 8 * (q_loc * 128 + p) + c
        idx_core = idx[b_loc, t_loc]  # [PT]
        m["idxs"] = np.ascontiguousarray(idx_core.reshape(NT, 128).T).astype(np.int32)
        m["posT"] = np.ascontiguousarray(pos_f[t_loc].T).astype(np.float32)
        mk = np.zeros((128, NCORES * 256), dtype=np.float32)
        for cp in range(NCORES):
            tri = np.triu(np.ones((128, 128), np.float32), 0 if cp <= c else 1)
            mk[:, cp * 256:(cp + 1) * 256] = np.tile(tri, (1, 2))
        m["masks"] = mk.astype(bf16)
        in_maps.append(m)
    return in_maps


_NC_CACHE = {}


def _get_nc(nb, L, V):
    key = (nb, L, V)
    if key not in _NC_CACHE:
        _NC_CACHE[key] = build_nc(nb, L, V)
    return _NC_CACHE[key]


def run_on_hw(nb, L, V, inputs, trace=False):
    from concourse import bass_utils
    nc = _get_nc(nb, L, V)
    in_maps = _prep_inputs(nb, L, V, **inputs)
    res = bass_utils.run_bass_kernel_spmd(nc, in_maps, core_ids=list(range(NCORES)),
                                          trace=trace)
    return res


def assemble(nb, L, V, results, lnf_b, tok_emb):
    T = 2048
    out = np.empty((2, T, V), dtype=np.float32)
    for c in range(NCORES):
        lg = results[c]["logits"].astype(np.float32).reshape(NT, 128, V)
        for tt in range(NT):
            b = tt // 2
            q = tt % 2
            t0 = 8 * (q * 128 + np.arange(128)) + c
            out[b, t0, :] = lg[tt]
    lnf_b = np.asarray(lnf_b, dtype=np.float32)
    if np.any(lnf_b):
        out += (lnf_b @ np.asarray(tok_emb, dtype=np.float32).T)[None, None, :]
    return out


def kernel(**inputs):
    nb, L, V = 2, 6, 32000
    res = run_on_hw(nb, L, V, inputs)
    return assemble(nb, L, V, res.results, inputs["lnf_b"], inputs["tok_emb"])
